# revision 28
# baseline (speedup 1.0000x reference)
"""GQA causal attention (B=1, S=4096, DIM=2048, 32 q-heads / 8 kv-heads,
head_dim 64) on 8 trn2 NeuronCores, tensor-parallel over heads.

Per-core dataflow (core i owns q-heads 4i..4i+3 and kv-head i), everything
kept in "transposed" layouts so no probability transposes are needed:
  phase 1: Q^T [256,S], K^T/V^T [128,S] projections from x^T (fp32r matmuls)
  phase 2: per (q-tile 512, head-pair): scores^T [k,q] = K^T.T @ Q^T chunks
           (two row-tiled K=64 matmuls), causal tile skipping, additive tri
           mask on diagonal blocks, ACT exp (scale=1/8 folded in), AV with a
           ones column appended to V so row 64 of the AV psum is the softmax
           denominator; divide via reciprocal_approx_fast + partition
           broadcast; out-proj emits out^T partials, host sums across cores.
"""

import sys
import numpy as np

sys.path.insert(0, "/opt/trn_rl_repo")

DIM = 2048
S = 4096
NH = 32
NKV = 8
HD = 64
NCORES = 8
HPC = NH // NCORES          # 4 q-heads per core
DQ = HPC * HD               # 256 q-proj cols per core
CC = DIM // 128             # 16 contraction chunks
QT = S // 512               # 8 q-tiles
KCN = S // 128              # 32 key chunks

_CACHE = {}


def _build_program():
    import concourse.bass as bass  # noqa: F401
    import concourse.mybir as mybir
    import concourse.tile as tile
    from concourse import bacc
    from concourse.masks import make_identity
    from contextlib import ExitStack

    f32 = mybir.dt.float32
    f32r = mybir.dt.float32r
    AF = mybir.ActivationFunctionType
    ALU = mybir.AluOpType

    bf16 = mybir.dt.bfloat16
    nc = bacc.Bacc()
    xT_d = nc.declare_dram_parameter("xT", [DIM, S], bf16, isOutput=False)
    wq_d = nc.declare_dram_parameter("wq", [DIM, DQ], bf16, isOutput=False)
    wkv_d = nc.declare_dram_parameter("wkv", [DIM, 128], bf16, isOutput=False)
    wo_d = nc.declare_dram_parameter("wo", [DQ, DIM], bf16, isOutput=False)
    tri_d = nc.declare_dram_parameter("tri", [128, 128], f32, isOutput=False)
    outT_d = nc.declare_dram_parameter("outT", [DIM, S], f32, isOutput=True)

    with tile.TileContext(nc) as tc, ExitStack() as ctx:
        const = ctx.enter_context(tc.tile_pool(name="const", bufs=1))
        wpool = ctx.enter_context(tc.tile_pool(name="wpool", bufs=1))
        big = ctx.enter_context(tc.tile_pool(name="big", bufs=1))
        xt_pool = ctx.enter_context(tc.tile_pool(name="xt", bufs=8))
        probs_pool = ctx.enter_context(tc.tile_pool(name="probs", bufs=6))
        slab_pool = ctx.enter_context(tc.tile_pool(name="slab", bufs=2))
        small = ctx.enter_context(tc.tile_pool(name="small", bufs=2))
        outsb = ctx.enter_context(tc.tile_pool(name="outsb", bufs=3))

        tri_sb = const.tile([128, 128], f32)
        nc.gpsimd.dma_start(tri_sb[:], tri_d[:, :])
        tri2_sb = const.tile([128, 2, 128], f32)
        nc.vector.tensor_copy(tri2_sb[:, 0, :], tri_sb[:])
        nc.vector.tensor_copy(tri2_sb[:, 1, :], tri_sb[:])
        ident = const.tile([128, 128], f32)
        make_identity(nc, ident[:])

        wq_sb = wpool.tile([128, CC, DQ], bf16)
        nc.gpsimd.dma_start(wq_sb[:], wq_d.rearrange("(o p) d -> p o d", p=128))
        wkv_sb = wpool.tile([128, CC, 128], bf16)
        nc.gpsimd.dma_start(wkv_sb[:], wkv_d.rearrange("(o p) d -> p o d", p=128))
        wo_sb = wpool.tile([128, 2, DIM], bf16)
        nc.gpsimd.dma_start(wo_sb[:], wo_d.rearrange("(o p) d -> p o d", p=128))

        q01_sb = big.tile([128, S], bf16)
        q23_sb = big.tile([128, S], bf16)
        kv_sb = big.tile([128, S], bf16)     # rows 0:64 K^T, 64:128 V^T
        kk2_sb = big.tile([128, S], bf16)    # rows 64:128 = copy of K^T
        xv_sb = big.tile([128, KCN, 65], bf16)  # V in [k, d] layout + ones col
        nc.vector.memset(xv_sb[:, :, 64:65], 1.0)

        # ------------- fused projections + attention -------------
        # Projections for s-tile j are emitted just before attention q-tile j
        # (causal attention only needs keys <= 512(j+1)), so projection
        # matmuls fill PE gaps in the ACT-paced attention pipeline and there
        # is no phase boundary. PSUM budget: proj 1 + op/vt 1 + sc2 2x2 +
        # av 2 = 8 banks.
        xT_r = xT_d.rearrange("(o p) s -> p o s", p=128)
        with tc.tile_pool(name="pssc", bufs=2, space="PSUM") as pssc, \
             tc.tile_pool(name="psav", bufs=1, space="PSUM") as psav, \
             tc.tile_pool(name="scratch", bufs=1, space="PSUM") as scratch:

            def emit_proj(s):
                sl = slice(512 * s, 512 * (s + 1))
                xts_l = []
                for half in range(4):
                    xts = xt_pool.tile([128, 4, 512], bf16, name="xts")
                    nc.sync.dma_start(xts[:], xT_r[:, 4 * half:4 * half + 4, sl])
                    xts_l.append(xts)

                def mm_pass(w_ap_fn, out_cb):
                    acc = scratch.tile([128, 512], f32, name="proj")
                    for c in range(CC):
                        nc.tensor.matmul(acc[:], w_ap_fn(c), xts_l[c // 4][:, c % 4, :],
                                         start=(c == 0), stop=(c == CC - 1))
                    out_cb(acc)

                # pass order: q01 first (head pair 0 needs it), then K/V
                # (+ transposes), then q23
                mm_pass(lambda c: wq_sb[:, c, 0:128],
                        lambda acc: nc.vector.tensor_copy(q01_sb[:, sl], acc[:]))

                def kv_out(acc):
                    nc.vector.tensor_copy(kv_sb[:, sl], acc[:])
                    nc.vector.tensor_copy(kk2_sb[64:128, sl], acc[0:64, :])
                    vts = small.tile([64, 512], f32, name="vts")
                    nc.vector.tensor_copy(vts[:], acc[64:128, :])
                    for t in range(4):
                        kc = 4 * s + t
                        vt_ps = scratch.tile([128, 512], f32, name="opvt")
                        nc.tensor.transpose(
                            vt_ps[:, 0:64],
                            vts[0:64, 128 * t:128 * (t + 1)],
                            ident[0:64, 0:64],
                        )
                        nc.vector.tensor_copy(xv_sb[:, kc, 0:64], vt_ps[:, 0:64])
                mm_pass(lambda c: wkv_sb[:, c, :], kv_out)
                mm_pass(lambda c: wq_sb[:, c, 128:256],
                        lambda acc: nc.vector.tensor_copy(q23_sb[:, sl], acc[:]))

            def emit_outproj(slabs2, qsl2, e):
                esl = slice(128 * e, 128 * (e + 1))
                op = scratch.tile([128, 512], f32, name="opvt")
                nc.tensor.matmul(op[:], wo_sb[:, 0, esl], slabs2[0][:],
                                 start=True, stop=False)
                nc.tensor.matmul(op[:], wo_sb[:, 1, esl], slabs2[1][:],
                                 start=False, stop=True)
                ob = outsb.tile([128, 512], f32, name="ob")
                nc.vector.tensor_copy(ob[:], op[:])
                nc.sync.dma_start(outT_d[esl, qsl2], ob[:])

            prev = None  # (slabs, qsl) of previous q-tile, out-proj pending
            for j in range(QT):
                q0 = 512 * j
                qsl = slice(q0, q0 + 512)
                emit_proj(j)
                pend = list(range(DIM // 128)) if prev is not None else []
                slabs = []
                for hp in range(2):
                    qT = q01_sb if hp == 0 else q23_sb
                    avA = psav.tile([65, 512], f32, name="avA")
                    avB = psav.tile([65, 512], f32, name="avB")
                    nkc = 4 * j + 4

                    def emit_scores(kc):
                        qoff = max(0, 128 * kc - q0)
                        ksl = slice(128 * kc, 128 * (kc + 1))
                        sc2 = pssc.tile([128, 2, 512], f32, name="sc2")
                        nc.tensor.matmul(
                            sc2[:, 0, qoff:], kv_sb[0:64, ksl],
                            qT[0:64, 512 * j + qoff:512 * (j + 1)],
                            start=True, stop=True)
                        nc.tensor.matmul(
                            sc2[:, 1, qoff:], kk2_sb[64:128, ksl],
                            qT[64:128, 512 * j + qoff:512 * (j + 1)],
                            start=True, stop=True)
                        if kc >= 4 * j:  # diagonal-crossing chunk
                            nc.vector.tensor_tensor(
                                sc2[:, :, qoff:qoff + 128],
                                sc2[:, :, qoff:qoff + 128],
                                tri2_sb[:], ALU.add)
                        return sc2, qoff

                    # software-pipelined: scores(kc+1) is emitted before the
                    # AV of kc so the PE prefers feeding the exp chain
                    cur = emit_scores(0)
                    for kc in range(nkc):
                        sc2, qoff = cur
                        p2 = probs_pool.tile([128, 2, 512], bf16, name="p2")
                        nc.scalar.activation(p2[:, :, qoff:], sc2[:, :, qoff:],
                                             AF.Exp, bias=0.0, scale=0.125)
                        if kc + 1 < nkc:
                            cur = emit_scores(kc + 1)
                        st = dict(start=(kc == 0), stop=(kc == nkc - 1),
                                  skip_group_check=True)
                        nc.tensor.matmul(avA[:, qoff:], xv_sb[:, kc, :],
                                         p2[:, 0, qoff:], **st)
                        nc.tensor.matmul(avB[:, qoff:], xv_sb[:, kc, :],
                                         p2[:, 1, qoff:], **st)
                        # spread the previous q-tile's out-proj through this
                        # tile's attention
                        if kc % 3 == 0 and pend:
                            emit_outproj(prev[0], prev[1], pend.pop(0))
                    # free the av psum banks early with one copy, then divide
                    # from SBUF so the next head-pair's AV doesn't stall
                    avsA = small.tile([65, 512], f32, name="avsA")
                    avsB = small.tile([65, 512], f32, name="avsB")
                    nc.vector.tensor_copy(avsA[:], avA[:])
                    nc.vector.tensor_copy(avsB[:], avB[:])
                    zA = small.tile([1, 512], f32, name="zA")
                    zB = small.tile([1, 512], f32, name="zB")
                    nc.vector.tensor_copy(zA[:], avsA[64:65, :])
                    nc.vector.tensor_copy(zB[:], avsB[64:65, :])
                    rA = small.tile([1, 512], f32, name="rA")
                    rB = small.tile([1, 512], f32, name="rB")
                    nc.vector.reciprocal_approx_fast(out=rA[:], in_=zA[:])
                    nc.vector.reciprocal_approx_fast(out=rB[:], in_=zB[:])
                    rbcA = small.tile([64, 512], f32, name="rbcA")
                    rbcB = small.tile([64, 512], f32, name="rbcB")
                    nc.gpsimd.partition_broadcast(rbcA[:], rA[:])
                    nc.gpsimd.partition_broadcast(rbcB[:], rB[:])
                    slab = slab_pool.tile([128, 512], bf16, name=f"slab{hp}")
                    nc.vector.tensor_tensor(slab[0:64, :], avsA[0:64, :],
                                            rbcA[:], ALU.mult)
                    nc.vector.tensor_tensor(slab[64:128, :], avsB[0:64, :],
                                            rbcB[:], ALU.mult)
                    slabs.append(slab)
                for e in pend:  # any leftovers from the previous tile
                    emit_outproj(prev[0], prev[1], e)
                prev = (slabs, qsl)
            for e in range(DIM // 128):  # final q-tile's out-proj
                emit_outproj(prev[0], prev[1], e)

    nc.compile()
    return nc


def _get_program():
    if "nc" not in _CACHE:
        _CACHE["nc"] = _build_program()
    return _CACHE["nc"]


def run(inputs, trace=False):
    from concourse.bass_utils import run_bass_kernel_spmd
    import ml_dtypes

    nc = _get_program()
    x = np.asarray(inputs["x"], dtype=np.float32)
    mask = np.asarray(inputs["mask"], dtype=np.float32)
    wq = np.asarray(inputs["wq"], dtype=np.float32)
    wk = np.asarray(inputs["wk"], dtype=np.float32)
    wv = np.asarray(inputs["wv"], dtype=np.float32)
    wo = np.asarray(inputs["wo"], dtype=np.float32)

    xT = np.ascontiguousarray(x[0].T).astype(ml_dtypes.bfloat16)   # [DIM, S]
    tri = np.ascontiguousarray(mask[0, 0, :128, :128].T)  # [k, q] additive

    in_maps = []
    for i in range(NCORES):
        in_maps.append({
            "xT": xT,
            "wq": np.ascontiguousarray(
                wq[:, i * DQ:(i + 1) * DQ]).astype(ml_dtypes.bfloat16),
            "wkv": np.ascontiguousarray(np.concatenate(
                [wk[:, i * HD:(i + 1) * HD], wv[:, i * HD:(i + 1) * HD]],
                axis=1)).astype(ml_dtypes.bfloat16),
            "wo": np.ascontiguousarray(
                wo[i * DQ:(i + 1) * DQ, :]).astype(ml_dtypes.bfloat16),
            "tri": tri,
        })
    res = run_bass_kernel_spmd(nc, in_maps, list(range(NCORES)), trace=trace)
    outT = res.results[0]["outT"].copy()
    for i in range(1, NCORES):
        outT += res.results[i]["outT"]
    out = np.ascontiguousarray(outT.T).reshape(1, S, DIM)
    return out, res


def kernel(**inputs) -> np.ndarray:
    out, _ = run(inputs, trace=False)
    return out


# revision 29
# speedup vs baseline: 1.0166x; 1.0166x over previous
"""GQA causal attention (B=1, S=4096, DIM=2048, 32 q-heads / 8 kv-heads,
head_dim 64) on 8 trn2 NeuronCores, tensor-parallel over heads.

Per-core dataflow (core i owns q-heads 4i..4i+3 and kv-head i), everything
kept in "transposed" layouts so no probability transposes are needed:
  phase 1: Q^T [256,S], K^T/V^T [128,S] projections from x^T (fp32r matmuls)
  phase 2: per (q-tile 512, head-pair): scores^T [k,q] = K^T.T @ Q^T chunks
           (two row-tiled K=64 matmuls), causal tile skipping, additive tri
           mask on diagonal blocks, ACT exp (scale=1/8 folded in), AV with a
           ones column appended to V so row 64 of the AV psum is the softmax
           denominator; divide via reciprocal_approx_fast + partition
           broadcast; out-proj emits out^T partials, host sums across cores.
"""

import sys
import numpy as np

sys.path.insert(0, "/opt/trn_rl_repo")

DIM = 2048
S = 4096
NH = 32
NKV = 8
HD = 64
NCORES = 8
HPC = NH // NCORES          # 4 q-heads per core
DQ = HPC * HD               # 256 q-proj cols per core
CC = DIM // 128             # 16 contraction chunks
QT = S // 512               # 8 q-tiles
KCN = S // 128              # 32 key chunks

_CACHE = {}


def _build_program():
    import concourse.bass as bass  # noqa: F401
    import concourse.mybir as mybir
    import concourse.tile as tile
    from concourse import bacc
    from concourse.masks import make_identity
    from contextlib import ExitStack

    f32 = mybir.dt.float32
    f32r = mybir.dt.float32r
    AF = mybir.ActivationFunctionType
    ALU = mybir.AluOpType

    bf16 = mybir.dt.bfloat16
    nc = bacc.Bacc()
    xT_d = nc.declare_dram_parameter("xT", [DIM, S], bf16, isOutput=False)
    wq_d = nc.declare_dram_parameter("wq", [DIM, DQ], bf16, isOutput=False)
    wkv_d = nc.declare_dram_parameter("wkv", [DIM, 128], bf16, isOutput=False)
    wo_d = nc.declare_dram_parameter("wo", [DQ, DIM], bf16, isOutput=False)
    tri_d = nc.declare_dram_parameter("tri", [128, 128], f32, isOutput=False)
    outT_d = nc.declare_dram_parameter("outT", [DIM, S], f32, isOutput=True)

    with tile.TileContext(nc) as tc, ExitStack() as ctx:
        const = ctx.enter_context(tc.tile_pool(name="const", bufs=1))
        wpool = ctx.enter_context(tc.tile_pool(name="wpool", bufs=1))
        big = ctx.enter_context(tc.tile_pool(name="big", bufs=1))
        xt_pool = ctx.enter_context(tc.tile_pool(name="xt", bufs=8))
        probs_pool = ctx.enter_context(tc.tile_pool(name="probs", bufs=6))
        slab_pool = ctx.enter_context(tc.tile_pool(name="slab", bufs=2))
        small = ctx.enter_context(tc.tile_pool(name="small", bufs=2))
        outsb = ctx.enter_context(tc.tile_pool(name="outsb", bufs=3))

        tri_sb = const.tile([128, 128], f32)
        nc.gpsimd.dma_start(tri_sb[:], tri_d[:, :])
        tri2_sb = const.tile([128, 2, 128], f32)
        nc.vector.tensor_copy(tri2_sb[:, 0, :], tri_sb[:])
        nc.vector.tensor_copy(tri2_sb[:, 1, :], tri_sb[:])
        ident = const.tile([128, 128], f32)
        make_identity(nc, ident[:])

        wq_sb = wpool.tile([128, CC, DQ], bf16)
        nc.gpsimd.dma_start(wq_sb[:], wq_d.rearrange("(o p) d -> p o d", p=128))
        wkv_sb = wpool.tile([128, CC, 128], bf16)
        nc.gpsimd.dma_start(wkv_sb[:], wkv_d.rearrange("(o p) d -> p o d", p=128))
        wo_sb = wpool.tile([128, 2, DIM], bf16)
        nc.gpsimd.dma_start(wo_sb[:], wo_d.rearrange("(o p) d -> p o d", p=128))

        q01_sb = big.tile([128, S], bf16)
        q23_sb = big.tile([128, S], bf16)
        kv_sb = big.tile([128, S], bf16)     # rows 0:64 K^T, 64:128 V^T
        kk2_sb = big.tile([128, S], bf16)    # rows 64:128 = copy of K^T
        xv_sb = big.tile([128, KCN, 65], bf16)  # V in [k, d] layout + ones col
        nc.vector.memset(xv_sb[:, :, 64:65], 1.0)

        # ------------- fused projections + attention -------------
        # Projections for s-tile j are emitted just before attention q-tile j
        # (causal attention only needs keys <= 512(j+1)), so projection
        # matmuls fill PE gaps in the ACT-paced attention pipeline and there
        # is no phase boundary. PSUM budget: proj 1 + op/vt 1 + sc2 2x2 +
        # av 2 = 8 banks.
        xT_r = xT_d.rearrange("(o p) s -> p o s", p=128)
        with tc.tile_pool(name="pssc", bufs=2, space="PSUM") as pssc, \
             tc.tile_pool(name="psav", bufs=1, space="PSUM") as psav, \
             tc.tile_pool(name="scratch", bufs=1, space="PSUM") as scratch:

            def emit_proj(s):
                sl = slice(512 * s, 512 * (s + 1))
                xts_l = []
                for half in range(4):
                    xts = xt_pool.tile([128, 4, 512], bf16, name="xts")
                    nc.sync.dma_start(xts[:], xT_r[:, 4 * half:4 * half + 4, sl])
                    xts_l.append(xts)

                def mm_pass(w_ap_fn, out_cb):
                    acc = scratch.tile([128, 512], f32, name="proj")
                    for c in range(CC):
                        nc.tensor.matmul(acc[:], w_ap_fn(c), xts_l[c // 4][:, c % 4, :],
                                         start=(c == 0), stop=(c == CC - 1))
                    out_cb(acc)

                def q01_pass():
                    mm_pass(lambda c: wq_sb[:, c, 0:128],
                            lambda acc: nc.vector.tensor_copy(q01_sb[:, sl], acc[:]))

                def kv_out(acc):
                    nc.vector.tensor_copy(kv_sb[:, sl], acc[:])
                    nc.vector.tensor_copy(kk2_sb[64:128, sl], acc[0:64, :])
                    vts = small.tile([64, 512], f32, name="vts")
                    nc.vector.tensor_copy(vts[:], acc[64:128, :])
                    for t in range(4):
                        kc = 4 * s + t
                        vt_ps = scratch.tile([128, 512], f32, name="opvt")
                        nc.tensor.transpose(
                            vt_ps[:, 0:64],
                            vts[0:64, 128 * t:128 * (t + 1)],
                            ident[0:64, 0:64],
                        )
                        nc.vector.tensor_copy(xv_sb[:, kc, 0:64], vt_ps[:, 0:64])

                def kv_pass():
                    mm_pass(lambda c: wkv_sb[:, c, :], kv_out)

                def q23_pass():
                    mm_pass(lambda c: wq_sb[:, c, 128:256],
                            lambda acc: nc.vector.tensor_copy(q23_sb[:, sl], acc[:]))
                return q01_pass, kv_pass, q23_pass

            def emit_outproj(slabs2, qsl2, e):
                esl = slice(128 * e, 128 * (e + 1))
                op = scratch.tile([128, 512], f32, name="opvt")
                nc.tensor.matmul(op[:], wo_sb[:, 0, esl], slabs2[0][:],
                                 start=True, stop=False)
                nc.tensor.matmul(op[:], wo_sb[:, 1, esl], slabs2[1][:],
                                 start=False, stop=True)
                ob = outsb.tile([128, 512], f32, name="ob")
                nc.vector.tensor_copy(ob[:], op[:])
                nc.sync.dma_start(outT_d[esl, qsl2], ob[:])

            prev = None  # (slabs, qsl) of previous q-tile, out-proj pending
            for j in range(QT):
                q0 = 512 * j
                qsl = slice(q0, q0 + 512)
                q01_pass, kv_pass, q23_pass = emit_proj(j)
                q01_pass()
                passes = [kv_pass, q23_pass]
                pend = list(range(DIM // 128)) if prev is not None else []
                slabs = []
                for hp in range(2):
                    qT = q01_sb if hp == 0 else q23_sb
                    avA = psav.tile([65, 512], f32, name="avA")
                    avB = psav.tile([65, 512], f32, name="avB")
                    nkc = 4 * j + 4

                    def emit_scores(kc):
                        qoff = max(0, 128 * kc - q0)
                        ksl = slice(128 * kc, 128 * (kc + 1))
                        sc2 = pssc.tile([128, 2, 512], f32, name="sc2")
                        nc.tensor.matmul(
                            sc2[:, 0, qoff:], kv_sb[0:64, ksl],
                            qT[0:64, 512 * j + qoff:512 * (j + 1)],
                            start=True, stop=True)
                        nc.tensor.matmul(
                            sc2[:, 1, qoff:], kk2_sb[64:128, ksl],
                            qT[64:128, 512 * j + qoff:512 * (j + 1)],
                            start=True, stop=True)
                        if kc >= 4 * j:  # diagonal-crossing chunk
                            nc.vector.tensor_tensor(
                                sc2[:, :, qoff:qoff + 128],
                                sc2[:, :, qoff:qoff + 128],
                                tri2_sb[:], ALU.add)
                        return sc2, qoff

                    # software-pipelined: scores(kc+1) is emitted before the
                    # AV of kc so the PE prefers feeding the exp chain
                    if hp == 1 and passes:   # q23 needed before hp1 scores
                        for p in passes:
                            p()
                        passes = []
                    cur = emit_scores(0)
                    for kc in range(nkc):
                        sc2, qoff = cur
                        p2 = probs_pool.tile([128, 2, 512], bf16, name="p2")
                        nc.scalar.activation(p2[:, :, qoff:], sc2[:, :, qoff:],
                                             AF.Exp, bias=0.0, scale=0.125)
                        if kc + 2 >= 4 * j and passes:
                            # kv/xv of this s-tile needed by chunk 4j: emit the
                            # kv pass (and let q23 trail) before we get there
                            passes.pop(0)()
                        if kc + 1 < nkc:
                            cur = emit_scores(kc + 1)
                        st = dict(start=(kc == 0), stop=(kc == nkc - 1),
                                  skip_group_check=True)
                        nc.tensor.matmul(avA[:, qoff:], xv_sb[:, kc, :],
                                         p2[:, 0, qoff:], **st)
                        nc.tensor.matmul(avB[:, qoff:], xv_sb[:, kc, :],
                                         p2[:, 1, qoff:], **st)
                        # spread the previous q-tile's out-proj through this
                        # tile's attention
                        if kc % 3 == 0 and pend:
                            emit_outproj(prev[0], prev[1], pend.pop(0))
                    # free the av psum banks early with one copy, then divide
                    # from SBUF so the next head-pair's AV doesn't stall
                    avsA = small.tile([65, 512], f32, name="avsA")
                    avsB = small.tile([65, 512], f32, name="avsB")
                    nc.vector.tensor_copy(avsA[:], avA[:])
                    nc.vector.tensor_copy(avsB[:], avB[:])
                    zA = small.tile([1, 512], f32, name="zA")
                    zB = small.tile([1, 512], f32, name="zB")
                    nc.vector.tensor_copy(zA[:], avsA[64:65, :])
                    nc.vector.tensor_copy(zB[:], avsB[64:65, :])
                    rA = small.tile([1, 512], f32, name="rA")
                    rB = small.tile([1, 512], f32, name="rB")
                    nc.vector.reciprocal_approx_fast(out=rA[:], in_=zA[:])
                    nc.vector.reciprocal_approx_fast(out=rB[:], in_=zB[:])
                    rbcA = small.tile([64, 512], f32, name="rbcA")
                    rbcB = small.tile([64, 512], f32, name="rbcB")
                    nc.gpsimd.partition_broadcast(rbcA[:], rA[:])
                    nc.gpsimd.partition_broadcast(rbcB[:], rB[:])
                    slab = slab_pool.tile([128, 512], bf16, name=f"slab{hp}")
                    nc.vector.tensor_tensor(slab[0:64, :], avsA[0:64, :],
                                            rbcA[:], ALU.mult)
                    nc.vector.tensor_tensor(slab[64:128, :], avsB[0:64, :],
                                            rbcB[:], ALU.mult)
                    slabs.append(slab)
                for e in pend:  # any leftovers from the previous tile
                    emit_outproj(prev[0], prev[1], e)
                prev = (slabs, qsl)
            for e in range(DIM // 128):  # final q-tile's out-proj
                emit_outproj(prev[0], prev[1], e)

    nc.compile()
    return nc


def _get_program():
    if "nc" not in _CACHE:
        _CACHE["nc"] = _build_program()
    return _CACHE["nc"]


def run(inputs, trace=False):
    from concourse.bass_utils import run_bass_kernel_spmd
    import ml_dtypes

    nc = _get_program()
    x = np.asarray(inputs["x"], dtype=np.float32)
    mask = np.asarray(inputs["mask"], dtype=np.float32)
    wq = np.asarray(inputs["wq"], dtype=np.float32)
    wk = np.asarray(inputs["wk"], dtype=np.float32)
    wv = np.asarray(inputs["wv"], dtype=np.float32)
    wo = np.asarray(inputs["wo"], dtype=np.float32)

    xT = np.ascontiguousarray(x[0].T).astype(ml_dtypes.bfloat16)   # [DIM, S]
    tri = np.ascontiguousarray(mask[0, 0, :128, :128].T)  # [k, q] additive

    in_maps = []
    for i in range(NCORES):
        in_maps.append({
            "xT": xT,
            "wq": np.ascontiguousarray(
                wq[:, i * DQ:(i + 1) * DQ]).astype(ml_dtypes.bfloat16),
            "wkv": np.ascontiguousarray(np.concatenate(
                [wk[:, i * HD:(i + 1) * HD], wv[:, i * HD:(i + 1) * HD]],
                axis=1)).astype(ml_dtypes.bfloat16),
            "wo": np.ascontiguousarray(
                wo[i * DQ:(i + 1) * DQ, :]).astype(ml_dtypes.bfloat16),
            "tri": tri,
        })
    res = run_bass_kernel_spmd(nc, in_maps, list(range(NCORES)), trace=trace)
    outT = res.results[0]["outT"].copy()
    for i in range(1, NCORES):
        outT += res.results[i]["outT"]
    out = np.ascontiguousarray(outT.T).reshape(1, S, DIM)
    return out, res


def kernel(**inputs) -> np.ndarray:
    out, _ = run(inputs, trace=False)
    return out
